# revision 10
# baseline (speedup 1.0000x reference)
"""Trainium2 Bass kernel for nn_GCNNSingleKernel (gnn_message_passing).

Strategy:
- Data-parallel over batch B=16 across 8 NeuronCores (2 graphs per core).
- adj_mask (B,N,N) is outer(node,node) with node = diag(adj_mask); the host
  extracts the diagonal so the 64MB mask never moves to the device.
- Per graph everything stays on-chip. The (N,N) Gaussian kernel matrix is
  produced by a single matmul pass with two augmentation rows
  (T1=[emb;1;-sq/2], T2=[emb;-sq/2+BIGNEG*(node-1);1] so T1^T@T2 directly
  yields -d/2 plus a large negative offset on invalid-node columns) and one
  ACT Exp pass. Invalid-node columns die inside the exp, so no (N,N) mask
  multiply is ever needed.
- Features live in a gapped 128-partition layout [conv 0:48 | zeros | res
  64:112 | zeros] to satisfy the 32-partition alignment of engine APs;
  weights are host-padded to match. deg rides as an extra column of the
  adj@emb^T matmul. Final InstanceNorm + fcl + sigmoid run on-device.
"""
import sys
import numpy as np

sys.path.insert(0, '/opt/trn_rl_repo')

import concourse.bass as bass  # noqa: E402
import concourse.tile as tile  # noqa: E402
from concourse import mybir  # noqa: E402
from concourse.bass_utils import run_bass_kernel_spmd  # noqa: E402

AF = mybir.ActivationFunctionType
ALU = mybir.AluOpType
FP = mybir.dt.float32

B, F0, FM, N, L, H = 16, 16, 96, 1024, 4, 48
NC = 8          # cores
BPC = B // NC   # graphs per core
NT = N // 128   # 8 n-tiles
EPS = 1e-5


def _split_multi_waits(nc, maxw=1):
    """Walrus (CoreV3) rejects >1 sync-wait on one instruction; spread extras
    onto same-engine NoOps inserted just before."""
    for f in nc.m.functions:
        for bb in f.blocks:
            newlist, changed = [], False
            for inst in bb.instructions:
                si = getattr(inst, 'sync_info', None)
                if si is not None and si.on_wait and len(si.on_wait) > maxw:
                    waits = list(si.on_wait)
                    head, tail = waits[:-maxw], waits[-maxw:]
                    for k in range(0, len(head), maxw):
                        nop = mybir.InstNoOp(
                            name=f'{inst.name}-w{k}', ins=[], outs=[])
                        nop.engine = inst.engine
                        nop.sync_info = mybir.SyncInfo(
                            on_wait=head[k:k + maxw], on_update=[])
                        newlist.append(nop)
                    si.on_wait = tail
                    inst.sync_info = si
                    changed = True
                newlist.append(inst)
            if changed:
                bb.instructions = newlist


def build_program(alphas):
    """Per-core SPMD bass program. alphas: 4 floats baked as immediates."""
    nc = bass.Bass()

    emb_d = nc.dram_tensor('emb_in', [BPC, F0, N], FP, kind='ExternalInput')
    node_d = nc.dram_tensor('noderows', [BPC, 2, N], FP, kind='ExternalInput')
    ncol_d = nc.dram_tensor('node_colm', [BPC, 128, NT], FP, kind='ExternalInput')
    invnb_d = nc.dram_tensor('invnb', [BPC, 128, 1], FP, kind='ExternalInput')
    wc0_d = nc.dram_tensor('Wc0T', [3, F0, H], FP, kind='ExternalInput')
    wr0_d = nc.dram_tensor('Wr0T', [F0, H], FP, kind='ExternalInput')
    bc0_d = nc.dram_tensor('bc0', [H, 1], FP, kind='ExternalInput')
    br0_d = nc.dram_tensor('br0', [H, 1], FP, kind='ExternalInput')
    # gapped (128) lhsT chunks for cat1/cat2 + compact (96) chunk for cat3
    wc12_d = nc.dram_tensor('WcT12', [L - 1, 2, 128, H], FP, kind='ExternalInput')
    wc3_d = nc.dram_tensor('WcT3', [L - 1, FM, H], FP, kind='ExternalInput')
    wr_d = nc.dram_tensor('WrT', [L - 1, 128, H], FP, kind='ExternalInput')
    bc_d = nc.dram_tensor('bc', [L - 1, H, 1], FP, kind='ExternalInput')
    br_d = nc.dram_tensor('br', [L - 1, H, 1], FP, kind='ExternalInput')
    fclw_d = nc.dram_tensor('fclw', [128, 1], FP, kind='ExternalInput')
    id_d = nc.dram_tensor('ident', [128, 128], FP, kind='ExternalInput')
    scal_d = nc.dram_tensor('scal', [1, 4], FP, kind='ExternalInput')
    out_d = nc.dram_tensor('out', [BPC, 1], FP, kind='ExternalOutput')

    al = [float(a) for a in alphas]

    with tile.TileContext(nc) as tc:
        from contextlib import ExitStack
        with ExitStack() as ctx:
            const = ctx.enter_context(tc.tile_pool(name='const', bufs=1))
            p_raw = ctx.enter_context(tc.tile_pool(name='raw', bufs=2))
            p_nbc = ctx.enter_context(tc.tile_pool(name='nbc', bufs=2))
            p_T1 = ctx.enter_context(tc.tile_pool(name='T1', bufs=2))
            p_T2 = ctx.enter_context(tc.tile_pool(name='T2', bufs=2))
            p_s2 = ctx.enter_context(tc.tile_pool(name='s2', bufs=2))
            p_EN = ctx.enter_context(tc.tile_pool(name='EN', bufs=2))
            p_adj = ctx.enter_context(tc.tile_pool(name='adj', bufs=8))
            p_w = ctx.enter_context(tc.tile_pool(name='w', bufs=2))
            p_c2 = ctx.enter_context(tc.tile_pool(name='c2', bufs=2))
            p_c3 = ctx.enter_context(tc.tile_pool(name='c3', bufs=2))
            p_eo = ctx.enter_context(tc.tile_pool(name='eo', bufs=2))
            p_sm = ctx.enter_context(tc.tile_pool(name='sm', bufs=2))
            p_row = ctx.enter_context(tc.tile_pool(name='row', bufs=2))
            pg = ctx.enter_context(tc.tile_pool(name='ps_g', bufs=2, space='PSUM'))
            pp3 = ctx.enter_context(tc.tile_pool(name='ps_p3', bufs=2, space='PSUM'))
            pmi = ctx.enter_context(tc.tile_pool(name='ps_mi', bufs=4, space='PSUM'))

            # ---- constants ----
            id_sb = const.tile([128, 128], FP)
            nc.sync.dma_start(out=id_sb, in_=id_d[:, :])
            ones_col = const.tile([128, 1], FP)
            nc.vector.memset(ones_col, 1.0)
            ones_row = const.tile([1, 128], FP)
            nc.vector.memset(ones_row, 1.0)
            ones_rowN = const.tile([1, N], FP)
            nc.vector.memset(ones_rowN, 1.0)
            eps_col = const.tile([128, 1], FP)
            nc.vector.memset(eps_col, EPS)
            wc0_sb = const.tile([F0, 3, H], FP)
            for k in range(3):
                nc.sync.dma_start(out=wc0_sb[:, k, :], in_=wc0_d[k, :, :])
            wr0_sb = const.tile([F0, H], FP)
            nc.sync.dma_start(out=wr0_sb, in_=wr0_d[:, :])
            bc0_sb = const.tile([H, 1], FP)
            nc.sync.dma_start(out=bc0_sb, in_=bc0_d[:, :])
            br0_sb = const.tile([H, 1], FP)
            nc.sync.dma_start(out=br0_sb, in_=br0_d[:, :])
            wc12_sb = const.tile([128, L - 1, 2, H], FP)
            for ll in range(L - 1):
                for k in range(2):
                    nc.sync.dma_start(out=wc12_sb[:, ll, k, :],
                                      in_=wc12_d[ll, k, :, :])
            wc3_sb = const.tile([FM, L - 1, H], FP)
            for ll in range(L - 1):
                nc.sync.dma_start(out=wc3_sb[:, ll, :], in_=wc3_d[ll, :, :])
            wr_sb = const.tile([128, L - 1, H], FP)
            for ll in range(L - 1):
                nc.sync.dma_start(out=wr_sb[:, ll, :], in_=wr_d[ll, :, :])
            bc_sb = const.tile([H, L - 1], FP)
            for ll in range(L - 1):
                nc.sync.dma_start(out=bc_sb[:, ll:ll + 1], in_=bc_d[ll, :, :])
            br_sb = const.tile([H, L - 1], FP)
            for ll in range(L - 1):
                nc.sync.dma_start(out=br_sb[:, ll:ll + 1], in_=br_d[ll, :, :])
            fclw_sb = const.tile([128, 1], FP)
            nc.sync.dma_start(out=fclw_sb, in_=fclw_d[:, :])
            scal_sb = const.tile([1, 4], FP)
            nc.sync.dma_start(out=scal_sb, in_=scal_d[:, :])

            for b in range(BPC):
                # ---- per-graph loads ----
                raw = p_raw.tile([F0, N], FP, tag='raw')
                nc.sync.dma_start(out=raw, in_=emb_d[b, :, :])
                nrow = p_row.tile([1, N], FP, tag='nrow')
                nc.sync.dma_start(out=nrow, in_=node_d[b, 0:1, :])
                nm1 = p_row.tile([1, N], FP, tag='nm1')
                nc.sync.dma_start(out=nm1, in_=node_d[b, 1:2, :])
                ncol = p_sm.tile([128, NT], FP, tag='ncol')
                nc.sync.dma_start(out=ncol, in_=ncol_d[b, :, :])
                invnb = p_sm.tile([128, 1], FP, tag='invnb')
                nc.sync.dma_start(out=invnb, in_=invnb_d[b, :, :])

                # node broadcast (128, N) via K=1 matmuls
                nbc = p_nbc.tile([128, N], FP, tag='nbc')
                for c in range(2):
                    pb = pmi.tile([128, 512], FP, tag='pmi')
                    nc.tensor.matmul(pb, lhsT=ones_row[0:1, 0:128],
                                     rhs=nrow[:, c * 512:(c + 1) * 512],
                                     start=True, stop=True)
                    nc.vector.tensor_copy(nbc[:, c * 512:(c + 1) * 512], pb)

                emb = raw  # current-layer fn source
                for ll in range(L):
                    first = ll == 0
                    F = F0 if first else 128        # stored feature rows
                    KA = (F0 + 2) if first else 128  # gram contraction depth
                    a1 = F0 if first else 48         # aug row: ones/rB
                    a2 = F0 + 1 if first else 112    # aug row: rA/ones
                    alpha = al[ll]
                    c_l = 45.0 / alpha
                    do_norm = not first

                    T1 = p_T1.tile([KA, N], FP, tag='T1')
                    nc.vector.tensor_tensor(T1[0:F, :], emb,
                                            nbc[0:F, :], op=ALU.mult)
                    s2 = p_s2.tile([F, N], FP, tag='s2')
                    nc.vector.tensor_tensor(s2, T1[0:F, :], T1[0:F, :],
                                            op=ALU.mult)
                    T2 = p_T2.tile([KA, N], FP, tag='T2')
                    nc.vector.tensor_copy(T2[0:F, :], T1[0:F, :])

                    if do_norm:
                        s_col = p_sm.tile([128, 1], FP, tag='scol')
                        nc.vector.tensor_reduce(s_col, T1[0:F, :],
                                                axis=mybir.AxisListType.X,
                                                op=ALU.add)
                        q_col = p_sm.tile([128, 1], FP, tag='qcol')
                        nc.vector.tensor_reduce(q_col, s2,
                                                axis=mybir.AxisListType.X,
                                                op=ALU.add)

                    # aug rows: rA = -sq/2 ; rB = rA + c_l*(node-1)
                    nm1c = p_row.tile([1, N], FP, tag='nm1c')
                    nc.vector.tensor_scalar(nm1c, nm1, c_l, None, op0=ALU.mult)
                    rA = p_row.tile([1, N], FP, tag='rA')
                    rB = p_row.tile([1, N], FP, tag='rB')
                    for c in range(2):
                        sl = slice(c * 512, (c + 1) * 512)
                        pr = pmi.tile([1, 512], FP, tag='pmi')
                        nc.tensor.matmul(pr, lhsT=ones_col[0:F, 0:1],
                                         rhs=s2[:, sl], start=True, stop=True)
                        nc.vector.tensor_scalar(rA[:, sl], pr, -0.5, None,
                                                op0=ALU.mult)
                        nc.vector.tensor_tensor(rB[:, sl], rA[:, sl],
                                                nm1c[:, sl], op=ALU.add)
                    # DMA aug rows (DMA is partition-alignment-free)
                    nc.sync.dma_start(out=T1[a1:a1 + 1, :], in_=ones_rowN[:, :])
                    nc.sync.dma_start(out=T1[a2:a2 + 1, :], in_=rA[:, :])
                    nc.sync.dma_start(out=T2[a1:a1 + 1, :], in_=rB[:, :])
                    nc.sync.dma_start(out=T2[a2:a2 + 1, :], in_=ones_rowN[:, :])

                    if do_norm:
                        m = p_sm.tile([128, 1], FP, tag='m')
                        nc.vector.tensor_scalar_mul(m, s_col, invnb[:, 0:1])
                        qn = p_sm.tile([128, 1], FP, tag='qn')
                        nc.vector.tensor_scalar_mul(qn, q_col, invnb[:, 0:1])
                        mm = p_sm.tile([128, 1], FP, tag='mm')
                        nc.vector.tensor_mul(mm, m, m)
                        v = p_sm.tile([128, 1], FP, tag='v')
                        nc.vector.tensor_sub(v, qn, mm)
                        sd = p_sm.tile([128, 1], FP, tag='sd')
                        nc.scalar.activation(sd, v, AF.Sqrt,
                                             bias=eps_col[:, 0:1], scale=1.0)
                        inv = p_sm.tile([128, 1], FP, tag='inv')
                        nc.vector.reciprocal(inv, sd)
                        EN = p_EN.tile([128, N], FP, tag='EN')
                        nc.vector.tensor_scalar(EN, T1[0:128, :], m, inv,
                                                op0=ALU.subtract, op1=ALU.mult)
                        cat1 = EN
                        wsrc = EN
                    else:
                        cat1 = raw
                        wsrc = T1  # rows 0:16 = masked raw

                    # gram + exp -> adj tiles (adj[t] holds rows t*128..)
                    adj_t = []
                    for t in range(NT):
                        at = p_adj.tile([128, N], FP, tag='adj')
                        adj_t.append(at)
                        for c in range(2):
                            sl = slice(c * 512, (c + 1) * 512)
                            pgt = pg.tile([128, 512], FP, tag='pg')
                            nc.tensor.matmul(
                                pgt, lhsT=T1[:, t * 128:(t + 1) * 128],
                                rhs=T2[:, sl], start=True, stop=True)
                            nc.scalar.activation(at[:, sl], pgt, AF.Exp,
                                                 bias=0.0, scale=2.0 * alpha)

                    # w blocks (128, WK): transposed masked features + node col
                    WK = 33 if first else 97
                    w_sb = p_w.tile([128, NT * WK], FP, tag='w')
                    if first:
                        nc.vector.memset(w_sb[:, :], 0.0)  # zero filler cols
                    for t in range(NT):
                        tsl = slice(t * 128, (t + 1) * 128)
                        if first:
                            pt = pmi.tile([128, F0], FP, tag='pmi')
                            nc.tensor.transpose(pt, wsrc[0:F0, tsl],
                                                id_sb[0:F0, 0:F0])
                            nc.vector.tensor_copy(
                                w_sb[:, t * WK:t * WK + F0], pt)
                        else:
                            pt = pmi.tile([128, H], FP, tag='pmi')
                            nc.tensor.transpose(pt, wsrc[0:H, tsl],
                                                id_sb[0:H, 0:H])
                            nc.vector.tensor_scalar_mul(
                                w_sb[:, t * WK:t * WK + H], pt,
                                ncol[:, t:t + 1])
                            pt2 = pmi.tile([128, H], FP, tag='pmi')
                            nc.tensor.transpose(pt2, wsrc[64:112, tsl],
                                                id_sb[64:112, 0:H])
                            nc.vector.tensor_scalar_mul(
                                w_sb[:, t * WK + H:t * WK + FM], pt2,
                                ncol[:, t:t + 1])
                        nc.vector.tensor_copy(
                            w_sb[:, t * WK + WK - 1:t * WK + WK],
                            ncol[:, t:t + 1])

                    # part3 (w^T @ adj) -> cat3 + deg row
                    NF3 = F0 if first else FM
                    PP = 33 if first else 97   # psum partitions (aligned reads)
                    cat3 = p_c3.tile([NF3, N], FP, tag='c3')
                    degrow = p_row.tile([1, N], FP, tag='degrow')
                    for c in range(2):
                        sl = slice(c * 512, (c + 1) * 512)
                        pp = pp3.tile([PP, 512], FP, tag='pp3')
                        for t in range(NT):
                            nc.tensor.matmul(
                                pp, lhsT=w_sb[:, t * WK:(t + 1) * WK],
                                rhs=adj_t[t][:, sl],
                                start=(t == 0), stop=(t == NT - 1))
                        nc.scalar.copy(cat3[:, sl], pp[0:NF3, :])
                        nc.scalar.copy(degrow[:, sl], pp[WK - 1:WK, :])

                    # cat2 = cat1 * deg (broadcast deg via K=1 matmul)
                    FC = F0 if first else 128
                    cat2 = p_c2.tile([FC, N], FP, tag='c2')
                    for c in range(2):
                        sl = slice(c * 512, (c + 1) * 512)
                        pd = pmi.tile([FC, 512], FP, tag='pmi')
                        nc.tensor.matmul(pd, lhsT=ones_row[0:1, 0:FC],
                                         rhs=degrow[:, sl],
                                         start=True, stop=True)
                        nc.vector.tensor_tensor(cat2[:, sl], cat1[:, sl],
                                                pd, op=ALU.mult)

                    # conv + res -> emb_out (gapped layout)
                    emb_out = p_eo.tile([128, N], FP, tag='eo')
                    nc.vector.memset(emb_out[32:64, :], 0.0)
                    nc.vector.memset(emb_out[96:128, :], 0.0)
                    for c in range(2):
                        sl = slice(c * 512, (c + 1) * 512)
                        pc_ = pmi.tile([H, 512], FP, tag='pmi')
                        for k in range(3):
                            if first:
                                lhsT = wc0_sb[:, k, :]
                            elif k < 2:
                                lhsT = wc12_sb[:, ll - 1, k, :]
                            else:
                                lhsT = wc3_sb[:, ll - 1, :]
                            nc.tensor.matmul(pc_, lhsT=lhsT,
                                             rhs=[cat1, cat2, cat3][k][:, sl],
                                             start=(k == 0), stop=(k == 2))
                        nc.scalar.activation(
                            emb_out[0:H, sl], pc_, AF.Relu,
                            bias=(bc0_sb[:, 0:1] if first
                                  else bc_sb[:, ll - 1:ll]), scale=1.0)
                        pr_ = pmi.tile([H, 512], FP, tag='pmi')
                        nc.tensor.matmul(
                            pr_, lhsT=(wr0_sb if first
                                       else wr_sb[:, ll - 1, :]),
                            rhs=cat1[:, sl], start=True, stop=True)
                        nc.scalar.activation(
                            emb_out[64:112, sl], pr_, AF.Identity,
                            bias=(br0_sb[:, 0:1] if first
                                  else br_sb[:, ll - 1:ll]), scale=1.0)
                    emb = emb_out

                # ---- epilogue ----
                fm = p_s2.tile([128, N], FP, tag='s2')
                pooled = p_sm.tile([128, 1], FP, tag='pooled')
                nc.vector.tensor_tensor(fm, emb, nbc, op=ALU.mult)
                nc.vector.tensor_reduce(pooled, fm,
                                        axis=mybir.AxisListType.X, op=ALU.add)
                p2 = p_sm.tile([128, 1], FP, tag='p2')
                nc.vector.tensor_mul(p2, pooled, pooled)
                pair = p_sm.tile([128, 2], FP, tag='pair')
                nc.vector.tensor_copy(pair[:, 0:1], pooled)
                nc.vector.tensor_copy(pair[:, 1:2], p2)
                psA = pmi.tile([1, 2], FP, tag='pmi')
                nc.tensor.matmul(psA, lhsT=ones_col[:, 0:1], rhs=pair,
                                 start=True, stop=True)
                psB = pmi.tile([1, 1], FP, tag='pmi')
                nc.tensor.matmul(psB, lhsT=fclw_sb, rhs=pooled,
                                 start=True, stop=True)
                sc = p_sm.tile([1, 8], FP, tag='sc')
                nc.vector.tensor_scalar(sc[:, 0:1], psA[:, 0:1], 1.0 / FM,
                                        None, op0=ALU.mult)   # mbar
                nc.vector.tensor_scalar(sc[:, 1:2], psA[:, 1:2], 1.0 / FM,
                                        None, op0=ALU.mult)   # qbar
                nc.vector.tensor_mul(sc[:, 2:3], sc[:, 0:1], sc[:, 0:1])
                nc.vector.tensor_sub(sc[:, 3:4], sc[:, 1:2], sc[:, 2:3])  # v
                nc.scalar.activation(sc[:, 4:5], sc[:, 3:4], AF.Sqrt,
                                     bias=eps_col[0:1, 0:1], scale=1.0)
                nc.vector.reciprocal(sc[:, 5:6], sc[:, 4:5])  # rv
                nc.vector.tensor_scalar_mul(sc[:, 6:7], sc[:, 0:1],
                                            scal_sb[:, 0:1])
                nc.vector.tensor_sub(sc[:, 7:8], psB[:, 0:1], sc[:, 6:7])
                sc2 = p_sm.tile([1, 2], FP, tag='sc2')
                nc.vector.tensor_mul(sc2[:, 0:1], sc[:, 7:8], sc[:, 5:6])
                nc.vector.tensor_scalar(sc2[:, 1:2], sc2[:, 0:1],
                                        scal_sb[:, 1:2], None, op0=ALU.add)
                outsb = p_sm.tile([1, 1], FP, tag='outsb')
                nc.scalar.activation(outsb, sc2[:, 1:2], AF.Sigmoid,
                                     bias=0.0, scale=1.0)
                nc.sync.dma_start(out=out_d[b:b + 1, :], in_=outsb)

    _split_multi_waits(nc)
    return nc


_CACHE = {}


def _get_program(alphas):
    key = tuple(float(a) for a in alphas)
    if key not in _CACHE:
        _CACHE[key] = build_program(alphas)
    return _CACHE[key]


def _gap(a96, axis=0):
    """Gapped-128 feature layout: [0:48]=f[0:48], [64:112]=f[48:96]."""
    shp = list(a96.shape)
    shp[axis] = 128
    out = np.zeros(shp, a96.dtype)
    idx0 = [slice(None)] * a96.ndim
    idx1 = [slice(None)] * a96.ndim
    src0 = [slice(None)] * a96.ndim
    src1 = [slice(None)] * a96.ndim
    idx0[axis] = slice(0, 48); src0[axis] = slice(0, 48)
    idx1[axis] = slice(64, 112); src1[axis] = slice(48, 96)
    out[tuple(idx0)] = a96[tuple(src0)]
    out[tuple(idx1)] = a96[tuple(src1)]
    return out


def kernel(**inputs):
    ins = {k: np.asarray(v) for k, v in inputs.items()}
    emb_in = ins['emb_in'].astype(np.float32)
    adj_mask = ins['adj_mask']
    nb = ins['batch_nb_nodes'].astype(np.float64)
    alphas = ins['alphas'].astype(np.float32)

    node = np.ascontiguousarray(
        np.einsum('bii->bi', adj_mask)).astype(np.float32)       # (B,N)
    noderows = np.ascontiguousarray(
        np.stack([node, node - 1.0], axis=1))                    # (B,2,N)
    node_colm = np.ascontiguousarray(
        node.reshape(B, NT, 128).transpose(0, 2, 1))             # (B,128,NT)
    invnb = np.ascontiguousarray(np.repeat(
        (1.0 / nb).astype(np.float32)[:, None], 128, axis=1)[:, :, None])

    Wc0T = np.ascontiguousarray(ins['Wc0'].astype(np.float32).T)  # (48,48)
    wc0 = np.ascontiguousarray(Wc0T.reshape(3, F0, H))
    wr0 = np.ascontiguousarray(ins['Wr0'].astype(np.float32).T)
    bc0 = ins['bc0'].astype(np.float32).reshape(H, 1)
    br0 = ins['br0'].astype(np.float32).reshape(H, 1)
    # layer >=1: Wc[l] (48,288) -> T (288,48) -> 3 chunks (96,48);
    # chunks 0,1 (cat1/cat2) padded to gapped 128 rows; chunk 2 compact.
    wc12 = np.zeros((L - 1, 2, 128, H), np.float32)
    wc3 = np.zeros((L - 1, FM, H), np.float32)
    wr = np.zeros((L - 1, 128, H), np.float32)
    for i in range(L - 1):
        WcT = ins['Wc'][i].astype(np.float32).T        # (288,48)
        wc12[i, 0] = _gap(WcT[0:96])
        wc12[i, 1] = _gap(WcT[96:192])
        wc3[i] = WcT[192:288]
        wr[i] = _gap(ins['Wr'][i].astype(np.float32).T)
    bc = ins['bc'].astype(np.float32).reshape(L - 1, H, 1)
    br = ins['br'].astype(np.float32).reshape(L - 1, H, 1)
    fclw = _gap(ins['fcl_w'].astype(np.float32).reshape(FM, 1))
    ident = np.zeros((128, 128), np.float32)
    ident[:64, :64] = np.eye(64)
    ident[64:112, 0:48] = np.eye(48)
    scal = np.array([[float(ins['fcl_w'].sum()),
                      float(ins['fcl_b'].reshape(-1)[0]), 0.0, 0.0]],
                    np.float32)

    nc_prog = _get_program(alphas)

    in_maps = []
    for k in range(NC):
        s = slice(k * BPC, (k + 1) * BPC)
        in_maps.append({
            'emb_in': np.ascontiguousarray(emb_in[s]),
            'noderows': np.ascontiguousarray(noderows[s]),
            'node_colm': np.ascontiguousarray(node_colm[s]),
            'invnb': np.ascontiguousarray(invnb[s]),
            'Wc0T': wc0, 'Wr0T': wr0, 'bc0': bc0, 'br0': br0,
            'WcT12': wc12, 'WcT3': wc3, 'WrT': wr, 'bc': bc, 'br': br,
            'fclw': fclw, 'ident': ident, 'scal': scal,
        })

    res = run_bass_kernel_spmd(nc_prog, in_maps, list(range(NC)))
    out = np.concatenate([res.results[k]['out'].reshape(BPC)
                          for k in range(NC)])
    return out.astype(np.float32)


if __name__ == '__main__':
    sys.path.insert(0, '/root/problem')
    import jax
    import reference as R
    cpu = jax.devices('cpu')[0]
    with jax.default_device(cpu):
        inp = {k: np.asarray(v) for k, v in R.setup_inputs().items()}
        exp = np.asarray(R.reference(**R.setup_inputs()))
    got = kernel(**inp)
    rel = np.abs(got - exp) / (np.abs(exp) + 1e-9)
    print('expected:', exp[:8])
    print('got     :', got[:8])
    print('max rel err:', rel.max())


# revision 13
# speedup vs baseline: 165.1643x; 165.1643x over previous
"""Trainium2 Bass kernel for nn_GCNNSingleKernel (gnn_message_passing).

Strategy:
- Data-parallel over batch B=16 across 8 NeuronCores (2 graphs per core).
- adj_mask (B,N,N) is outer(node,node) with node = diag(adj_mask); the host
  extracts the diagonal so the 64MB mask never moves to the device.
- Per graph everything stays on-chip. The (N,N) Gaussian kernel matrix is
  produced by a single matmul pass with two augmentation rows
  (T1=[emb;1;-sq/2], T2=[emb;-sq/2+BIGNEG*(node-1);1] so T1^T@T2 directly
  yields -d/2 plus a large negative offset on invalid-node columns) and one
  ACT Exp pass. Invalid-node columns die inside the exp, so no (N,N) mask
  multiply is ever needed.
- Features live in a gapped 128-partition layout [conv 0:48 | zeros | res
  64:112 | zeros] to satisfy the 32-partition alignment of engine APs;
  weights are host-padded to match. deg rides as an extra column of the
  adj@emb^T matmul. Final InstanceNorm + fcl + sigmoid run on-device.
"""
import sys
import numpy as np

sys.path.insert(0, '/opt/trn_rl_repo')

import concourse.bass as bass  # noqa: E402
import concourse.tile as tile  # noqa: E402
from concourse import mybir  # noqa: E402
from concourse.bass_utils import run_bass_kernel_spmd  # noqa: E402

AF = mybir.ActivationFunctionType
ALU = mybir.AluOpType
FP = mybir.dt.float32

B, F0, FM, N, L, H = 16, 16, 96, 1024, 4, 48
NC = 8          # cores
BPC = B // NC   # graphs per core
NT = N // 128   # 8 n-tiles
EPS = 1e-5


def _split_multi_waits(nc, maxw=1):
    """Walrus (CoreV3) rejects >1 sync-wait on one instruction; spread extras
    onto same-engine NoOps inserted just before."""
    for f in nc.m.functions:
        for bb in f.blocks:
            newlist, changed = [], False
            for inst in bb.instructions:
                si = getattr(inst, 'sync_info', None)
                if si is not None and si.on_wait and len(si.on_wait) > maxw:
                    waits = list(si.on_wait)
                    head, tail = waits[:-maxw], waits[-maxw:]
                    for k in range(0, len(head), maxw):
                        nop = mybir.InstNoOp(
                            name=f'{inst.name}-w{k}', ins=[], outs=[])
                        nop.engine = inst.engine
                        nop.sync_info = mybir.SyncInfo(
                            on_wait=head[k:k + maxw], on_update=[])
                        newlist.append(nop)
                    si.on_wait = tail
                    inst.sync_info = si
                    changed = True
                newlist.append(inst)
            if changed:
                bb.instructions = newlist


def build_program(alphas, reps=1):
    """Per-core SPMD bass program. alphas: 4 floats baked as immediates.
    reps>1 repeats the whole computation (timing variant)."""
    nc = bass.Bass()

    emb_d = nc.dram_tensor('emb_in', [BPC, F0, N], FP, kind='ExternalInput')
    node_d = nc.dram_tensor('noderows', [BPC, 2, N], FP, kind='ExternalInput')
    ncol_d = nc.dram_tensor('node_colm', [BPC, 128, NT], FP, kind='ExternalInput')
    invnb_d = nc.dram_tensor('invnb', [BPC, 128, 1], FP, kind='ExternalInput')
    wc0_d = nc.dram_tensor('Wc0T', [3, F0, H], FP, kind='ExternalInput')
    wr0_d = nc.dram_tensor('Wr0T', [F0, H], FP, kind='ExternalInput')
    bc0_d = nc.dram_tensor('bc0', [H, 1], FP, kind='ExternalInput')
    br0_d = nc.dram_tensor('br0', [H, 1], FP, kind='ExternalInput')
    # gapped (128) lhsT chunks for cat1/cat2 + compact (96) chunk for cat3
    wc12_d = nc.dram_tensor('WcT12', [L - 1, 2, 128, H], FP, kind='ExternalInput')
    wc3_d = nc.dram_tensor('WcT3', [L - 1, FM, H], FP, kind='ExternalInput')
    wr_d = nc.dram_tensor('WrT', [L - 1, 128, H], FP, kind='ExternalInput')
    bc_d = nc.dram_tensor('bc', [L - 1, H, 1], FP, kind='ExternalInput')
    br_d = nc.dram_tensor('br', [L - 1, H, 1], FP, kind='ExternalInput')
    fclw_d = nc.dram_tensor('fclw', [128, 1], FP, kind='ExternalInput')
    id_d = nc.dram_tensor('ident', [128, 128], FP, kind='ExternalInput')
    scal_d = nc.dram_tensor('scal', [1, 4], FP, kind='ExternalInput')
    out_d = nc.dram_tensor('out', [BPC, 1], FP, kind='ExternalOutput')

    al = [float(a) for a in alphas]

    with tile.TileContext(nc) as tc:
        from contextlib import ExitStack
        with ExitStack() as ctx:
            const = ctx.enter_context(tc.tile_pool(name='const', bufs=1))
            p_raw = ctx.enter_context(tc.tile_pool(name='raw', bufs=2))
            p_nbc = ctx.enter_context(tc.tile_pool(name='nbc', bufs=2))
            p_T1 = ctx.enter_context(tc.tile_pool(name='T1', bufs=2))
            p_T2 = ctx.enter_context(tc.tile_pool(name='T2', bufs=2))
            p_s2 = ctx.enter_context(tc.tile_pool(name='s2', bufs=2))
            p_EN = ctx.enter_context(tc.tile_pool(name='EN', bufs=2))
            p_adj = ctx.enter_context(tc.tile_pool(name='adj', bufs=8))
            p_w = ctx.enter_context(tc.tile_pool(name='w', bufs=2))
            p_c2 = ctx.enter_context(tc.tile_pool(name='c2', bufs=2))
            p_c3 = ctx.enter_context(tc.tile_pool(name='c3', bufs=2))
            p_eo = ctx.enter_context(tc.tile_pool(name='eo', bufs=2))
            p_sm = ctx.enter_context(tc.tile_pool(name='sm', bufs=2))
            p_row = ctx.enter_context(tc.tile_pool(name='row', bufs=2))
            pg = ctx.enter_context(tc.tile_pool(name='ps_g', bufs=2, space='PSUM'))
            pp3 = ctx.enter_context(tc.tile_pool(name='ps_p3', bufs=2, space='PSUM'))
            pmi = ctx.enter_context(tc.tile_pool(name='ps_mi', bufs=4, space='PSUM'))

            # ---- constants ----
            id_sb = const.tile([128, 128], FP)
            nc.sync.dma_start(out=id_sb, in_=id_d[:, :])
            ones_col = const.tile([128, 1], FP)
            nc.vector.memset(ones_col, 1.0)
            ones_row = const.tile([1, 128], FP)
            nc.vector.memset(ones_row, 1.0)
            ones_rowN = const.tile([1, N], FP)
            nc.vector.memset(ones_rowN, 1.0)
            eps_col = const.tile([128, 1], FP)
            nc.vector.memset(eps_col, EPS)
            wc0_sb = const.tile([F0, 3, H], FP)
            for k in range(3):
                nc.sync.dma_start(out=wc0_sb[:, k, :], in_=wc0_d[k, :, :])
            wr0_sb = const.tile([F0, H], FP)
            nc.sync.dma_start(out=wr0_sb, in_=wr0_d[:, :])
            bc0_sb = const.tile([H, 1], FP)
            nc.sync.dma_start(out=bc0_sb, in_=bc0_d[:, :])
            br0_sb = const.tile([H, 1], FP)
            nc.sync.dma_start(out=br0_sb, in_=br0_d[:, :])
            wc12_sb = const.tile([128, L - 1, 2, H], FP)
            for ll in range(L - 1):
                for k in range(2):
                    nc.sync.dma_start(out=wc12_sb[:, ll, k, :],
                                      in_=wc12_d[ll, k, :, :])
            wc3_sb = const.tile([FM, L - 1, H], FP)
            for ll in range(L - 1):
                nc.sync.dma_start(out=wc3_sb[:, ll, :], in_=wc3_d[ll, :, :])
            wr_sb = const.tile([128, L - 1, H], FP)
            for ll in range(L - 1):
                nc.sync.dma_start(out=wr_sb[:, ll, :], in_=wr_d[ll, :, :])
            bc_sb = const.tile([H, L - 1], FP)
            for ll in range(L - 1):
                nc.sync.dma_start(out=bc_sb[:, ll:ll + 1], in_=bc_d[ll, :, :])
            br_sb = const.tile([H, L - 1], FP)
            for ll in range(L - 1):
                nc.sync.dma_start(out=br_sb[:, ll:ll + 1], in_=br_d[ll, :, :])
            fclw_sb = const.tile([128, 1], FP)
            nc.sync.dma_start(out=fclw_sb, in_=fclw_d[:, :])
            scal_sb = const.tile([1, 4], FP)
            nc.sync.dma_start(out=scal_sb, in_=scal_d[:, :])

            for b in [bb_ for _ in range(reps) for bb_ in range(BPC)]:
                # ---- per-graph loads ----
                raw = p_raw.tile([F0, N], FP, tag='raw')
                nc.sync.dma_start(out=raw, in_=emb_d[b, :, :])
                nrow = p_row.tile([1, N], FP, tag='nrow')
                nc.sync.dma_start(out=nrow, in_=node_d[b, 0:1, :])
                nm1 = p_row.tile([1, N], FP, tag='nm1')
                nc.sync.dma_start(out=nm1, in_=node_d[b, 1:2, :])
                ncol = p_sm.tile([128, NT], FP, tag='ncol')
                nc.sync.dma_start(out=ncol, in_=ncol_d[b, :, :])
                invnb = p_sm.tile([128, 1], FP, tag='invnb')
                nc.sync.dma_start(out=invnb, in_=invnb_d[b, :, :])

                # node broadcast (128, N) via K=1 matmuls
                nbc = p_nbc.tile([128, N], FP, tag='nbc')
                for c in range(2):
                    pb = pmi.tile([128, 512], FP, tag='pmi')
                    nc.tensor.matmul(pb, lhsT=ones_row[0:1, 0:128],
                                     rhs=nrow[:, c * 512:(c + 1) * 512],
                                     start=True, stop=True)
                    nc.vector.tensor_copy(nbc[:, c * 512:(c + 1) * 512], pb)

                emb = raw  # current-layer fn source
                for ll in range(L):
                    first = ll == 0
                    F = F0 if first else 128        # stored feature rows
                    KA = (F0 + 2) if first else 128  # gram contraction depth
                    a1 = F0 if first else 48         # aug row: ones/rB
                    a2 = F0 + 1 if first else 112    # aug row: rA/ones
                    alpha = al[ll]
                    c_l = 45.0 / alpha
                    do_norm = not first

                    T1 = p_T1.tile([KA, N], FP, tag='T1')
                    nc.vector.tensor_tensor(T1[0:F, :], emb,
                                            nbc[0:F, :], op=ALU.mult)
                    s2 = p_s2.tile([F, N], FP, tag='s2')
                    nc.vector.tensor_tensor(s2, T1[0:F, :], T1[0:F, :],
                                            op=ALU.mult)
                    T2 = p_T2.tile([KA, N], FP, tag='T2')
                    nc.vector.tensor_copy(T2[0:F, :], T1[0:F, :])

                    if do_norm:
                        s_col = p_sm.tile([128, 1], FP, tag='scol')
                        nc.vector.tensor_reduce(s_col, T1[0:F, :],
                                                axis=mybir.AxisListType.X,
                                                op=ALU.add)
                        q_col = p_sm.tile([128, 1], FP, tag='qcol')
                        nc.vector.tensor_reduce(q_col, s2,
                                                axis=mybir.AxisListType.X,
                                                op=ALU.add)

                    # aug rows: rA = -sq/2 ; rB = rA + c_l*(node-1)
                    nm1c = p_row.tile([1, N], FP, tag='nm1c')
                    nc.vector.tensor_scalar(nm1c, nm1, c_l, None, op0=ALU.mult)
                    rA = p_row.tile([1, N], FP, tag='rA')
                    rB = p_row.tile([1, N], FP, tag='rB')
                    for c in range(2):
                        sl = slice(c * 512, (c + 1) * 512)
                        pr = pmi.tile([1, 512], FP, tag='pmi')
                        nc.tensor.matmul(pr, lhsT=ones_col[0:F, 0:1],
                                         rhs=s2[:, sl], start=True, stop=True)
                        nc.vector.tensor_scalar(rA[:, sl], pr, -0.5, None,
                                                op0=ALU.mult)
                        nc.vector.tensor_tensor(rB[:, sl], rA[:, sl],
                                                nm1c[:, sl], op=ALU.add)
                    # DMA aug rows (DMA is partition-alignment-free)
                    nc.sync.dma_start(out=T1[a1:a1 + 1, :], in_=ones_rowN[:, :])
                    nc.sync.dma_start(out=T1[a2:a2 + 1, :], in_=rA[:, :])
                    nc.sync.dma_start(out=T2[a1:a1 + 1, :], in_=rB[:, :])
                    nc.sync.dma_start(out=T2[a2:a2 + 1, :], in_=ones_rowN[:, :])

                    if do_norm:
                        m = p_sm.tile([128, 1], FP, tag='m')
                        nc.vector.tensor_scalar_mul(m, s_col, invnb[:, 0:1])
                        qn = p_sm.tile([128, 1], FP, tag='qn')
                        nc.vector.tensor_scalar_mul(qn, q_col, invnb[:, 0:1])
                        mm = p_sm.tile([128, 1], FP, tag='mm')
                        nc.vector.tensor_mul(mm, m, m)
                        v = p_sm.tile([128, 1], FP, tag='v')
                        nc.vector.tensor_sub(v, qn, mm)
                        sd = p_sm.tile([128, 1], FP, tag='sd')
                        nc.scalar.activation(sd, v, AF.Sqrt,
                                             bias=eps_col[:, 0:1], scale=1.0)
                        inv = p_sm.tile([128, 1], FP, tag='inv')
                        nc.vector.reciprocal(inv, sd)
                        EN = p_EN.tile([128, N], FP, tag='EN')
                        nc.vector.tensor_scalar(EN, T1[0:128, :], m, inv,
                                                op0=ALU.subtract, op1=ALU.mult)
                        cat1 = EN
                        wsrc = EN
                    else:
                        cat1 = raw
                        wsrc = T1  # rows 0:16 = masked raw

                    # gram + exp -> adj tiles (adj[t] holds rows t*128..)
                    adj_t = []
                    for t in range(NT):
                        at = p_adj.tile([128, N], FP, tag='adj')
                        adj_t.append(at)
                        for c in range(2):
                            sl = slice(c * 512, (c + 1) * 512)
                            pgt = pg.tile([128, 512], FP, tag='pg')
                            nc.tensor.matmul(
                                pgt, lhsT=T1[:, t * 128:(t + 1) * 128],
                                rhs=T2[:, sl], start=True, stop=True)
                            nc.scalar.activation(at[:, sl], pgt, AF.Exp,
                                                 bias=0.0, scale=2.0 * alpha)

                    # w blocks (128, WK): transposed masked features + node col
                    WK = 33 if first else 97
                    w_sb = p_w.tile([128, NT * WK], FP, tag='w')
                    if first:
                        nc.vector.memset(w_sb[:, :], 0.0)  # zero filler cols
                    for t in range(NT):
                        tsl = slice(t * 128, (t + 1) * 128)
                        if first:
                            pt = pmi.tile([128, F0], FP, tag='pmi')
                            nc.tensor.transpose(pt, wsrc[0:F0, tsl],
                                                id_sb[0:F0, 0:F0])
                            nc.vector.tensor_copy(
                                w_sb[:, t * WK:t * WK + F0], pt)
                        else:
                            pt = pmi.tile([128, H], FP, tag='pmi')
                            nc.tensor.transpose(pt, wsrc[0:H, tsl],
                                                id_sb[0:H, 0:H])
                            nc.vector.tensor_scalar_mul(
                                w_sb[:, t * WK:t * WK + H], pt,
                                ncol[:, t:t + 1])
                            pt2 = pmi.tile([128, H], FP, tag='pmi')
                            nc.tensor.transpose(pt2, wsrc[64:112, tsl],
                                                id_sb[64:112, 0:H])
                            nc.vector.tensor_scalar_mul(
                                w_sb[:, t * WK + H:t * WK + FM], pt2,
                                ncol[:, t:t + 1])
                        nc.vector.tensor_copy(
                            w_sb[:, t * WK + WK - 1:t * WK + WK],
                            ncol[:, t:t + 1])

                    # part3 (w^T @ adj) -> cat3 + deg row
                    NF3 = F0 if first else FM
                    PP = 33 if first else 97   # psum partitions (aligned reads)
                    cat3 = p_c3.tile([NF3, N], FP, tag='c3')
                    degrow = p_row.tile([1, N], FP, tag='degrow')
                    for c in range(2):
                        sl = slice(c * 512, (c + 1) * 512)
                        pp = pp3.tile([PP, 512], FP, tag='pp3')
                        for t in range(NT):
                            nc.tensor.matmul(
                                pp, lhsT=w_sb[:, t * WK:(t + 1) * WK],
                                rhs=adj_t[t][:, sl],
                                start=(t == 0), stop=(t == NT - 1))
                        nc.scalar.copy(cat3[:, sl], pp[0:NF3, :])
                        nc.scalar.copy(degrow[:, sl], pp[WK - 1:WK, :])

                    # cat2 = cat1 * deg (broadcast deg via K=1 matmul)
                    FC = F0 if first else 128
                    cat2 = p_c2.tile([FC, N], FP, tag='c2')
                    for c in range(2):
                        sl = slice(c * 512, (c + 1) * 512)
                        pd = pmi.tile([FC, 512], FP, tag='pmi')
                        nc.tensor.matmul(pd, lhsT=ones_row[0:1, 0:FC],
                                         rhs=degrow[:, sl],
                                         start=True, stop=True)
                        nc.vector.tensor_tensor(cat2[:, sl], cat1[:, sl],
                                                pd, op=ALU.mult)

                    # conv + res -> emb_out (gapped layout)
                    emb_out = p_eo.tile([128, N], FP, tag='eo')
                    nc.vector.memset(emb_out[32:64, :], 0.0)
                    nc.vector.memset(emb_out[96:128, :], 0.0)
                    for c in range(2):
                        sl = slice(c * 512, (c + 1) * 512)
                        pc_ = pmi.tile([H, 512], FP, tag='pmi')
                        for k in range(3):
                            if first:
                                lhsT = wc0_sb[:, k, :]
                            elif k < 2:
                                lhsT = wc12_sb[:, ll - 1, k, :]
                            else:
                                lhsT = wc3_sb[:, ll - 1, :]
                            nc.tensor.matmul(pc_, lhsT=lhsT,
                                             rhs=[cat1, cat2, cat3][k][:, sl],
                                             start=(k == 0), stop=(k == 2))
                        nc.scalar.activation(
                            emb_out[0:H, sl], pc_, AF.Relu,
                            bias=(bc0_sb[:, 0:1] if first
                                  else bc_sb[:, ll - 1:ll]), scale=1.0)
                        pr_ = pmi.tile([H, 512], FP, tag='pmi')
                        nc.tensor.matmul(
                            pr_, lhsT=(wr0_sb if first
                                       else wr_sb[:, ll - 1, :]),
                            rhs=cat1[:, sl], start=True, stop=True)
                        nc.scalar.activation(
                            emb_out[64:112, sl], pr_, AF.Identity,
                            bias=(br0_sb[:, 0:1] if first
                                  else br_sb[:, ll - 1:ll]), scale=1.0)
                    emb = emb_out

                # ---- epilogue ----
                fm = p_s2.tile([128, N], FP, tag='s2')
                pooled = p_sm.tile([128, 1], FP, tag='pooled')
                nc.vector.tensor_tensor(fm, emb, nbc, op=ALU.mult)
                nc.vector.tensor_reduce(pooled, fm,
                                        axis=mybir.AxisListType.X, op=ALU.add)
                p2 = p_sm.tile([128, 1], FP, tag='p2')
                nc.vector.tensor_mul(p2, pooled, pooled)
                pair = p_sm.tile([128, 2], FP, tag='pair')
                nc.vector.tensor_copy(pair[:, 0:1], pooled)
                nc.vector.tensor_copy(pair[:, 1:2], p2)
                psA = pmi.tile([1, 2], FP, tag='pmi')
                nc.tensor.matmul(psA, lhsT=ones_col[:, 0:1], rhs=pair,
                                 start=True, stop=True)
                psB = pmi.tile([1, 1], FP, tag='pmi')
                nc.tensor.matmul(psB, lhsT=fclw_sb, rhs=pooled,
                                 start=True, stop=True)
                sc = p_sm.tile([1, 8], FP, tag='sc')
                nc.vector.tensor_scalar(sc[:, 0:1], psA[:, 0:1], 1.0 / FM,
                                        None, op0=ALU.mult)   # mbar
                nc.vector.tensor_scalar(sc[:, 1:2], psA[:, 1:2], 1.0 / FM,
                                        None, op0=ALU.mult)   # qbar
                nc.vector.tensor_mul(sc[:, 2:3], sc[:, 0:1], sc[:, 0:1])
                nc.vector.tensor_sub(sc[:, 3:4], sc[:, 1:2], sc[:, 2:3])  # v
                nc.scalar.activation(sc[:, 4:5], sc[:, 3:4], AF.Sqrt,
                                     bias=eps_col[0:1, 0:1], scale=1.0)
                nc.vector.reciprocal(sc[:, 5:6], sc[:, 4:5])  # rv
                nc.vector.tensor_scalar_mul(sc[:, 6:7], sc[:, 0:1],
                                            scal_sb[:, 0:1])
                nc.vector.tensor_sub(sc[:, 7:8], psB[:, 0:1], sc[:, 6:7])
                sc2 = p_sm.tile([1, 2], FP, tag='sc2')
                nc.vector.tensor_mul(sc2[:, 0:1], sc[:, 7:8], sc[:, 5:6])
                nc.vector.tensor_scalar(sc2[:, 1:2], sc2[:, 0:1],
                                        scal_sb[:, 1:2], None, op0=ALU.add)
                outsb = p_sm.tile([1, 1], FP, tag='outsb')
                nc.scalar.activation(outsb, sc2[:, 1:2], AF.Sigmoid,
                                     bias=0.0, scale=1.0)
                nc.sync.dma_start(out=out_d[b:b + 1, :], in_=outsb)

    _split_multi_waits(nc)
    return nc


_CACHE = {}
_RUN_CACHE = {}
_LAST_INMAPS = None


def _get_program(alphas):
    key = tuple(float(a) for a in alphas)
    if key not in _CACHE:
        _CACHE[key] = build_program(alphas)
    return _CACHE[key]


def _gap(a96, axis=0):
    """Gapped-128 feature layout: [0:48]=f[0:48], [64:112]=f[48:96]."""
    shp = list(a96.shape)
    shp[axis] = 128
    out = np.zeros(shp, a96.dtype)
    idx0 = [slice(None)] * a96.ndim
    idx1 = [slice(None)] * a96.ndim
    src0 = [slice(None)] * a96.ndim
    src1 = [slice(None)] * a96.ndim
    idx0[axis] = slice(0, 48); src0[axis] = slice(0, 48)
    idx1[axis] = slice(64, 112); src1[axis] = slice(48, 96)
    out[tuple(idx0)] = a96[tuple(src0)]
    out[tuple(idx1)] = a96[tuple(src1)]
    return out


def kernel(**inputs):
    ins = {k: np.asarray(v) for k, v in inputs.items()}
    emb_in = ins['emb_in'].astype(np.float32)
    adj_mask = ins['adj_mask']
    nb = ins['batch_nb_nodes'].astype(np.float64)
    alphas = ins['alphas'].astype(np.float32)

    node = np.ascontiguousarray(
        np.einsum('bii->bi', adj_mask)).astype(np.float32)       # (B,N)
    noderows = np.ascontiguousarray(
        np.stack([node, node - 1.0], axis=1))                    # (B,2,N)
    node_colm = np.ascontiguousarray(
        node.reshape(B, NT, 128).transpose(0, 2, 1))             # (B,128,NT)
    invnb = np.ascontiguousarray(np.repeat(
        (1.0 / nb).astype(np.float32)[:, None], 128, axis=1)[:, :, None])

    Wc0T = np.ascontiguousarray(ins['Wc0'].astype(np.float32).T)  # (48,48)
    wc0 = np.ascontiguousarray(Wc0T.reshape(3, F0, H))
    wr0 = np.ascontiguousarray(ins['Wr0'].astype(np.float32).T)
    bc0 = ins['bc0'].astype(np.float32).reshape(H, 1)
    br0 = ins['br0'].astype(np.float32).reshape(H, 1)
    # layer >=1: Wc[l] (48,288) -> T (288,48) -> 3 chunks (96,48);
    # chunks 0,1 (cat1/cat2) padded to gapped 128 rows; chunk 2 compact.
    wc12 = np.zeros((L - 1, 2, 128, H), np.float32)
    wc3 = np.zeros((L - 1, FM, H), np.float32)
    wr = np.zeros((L - 1, 128, H), np.float32)
    for i in range(L - 1):
        WcT = ins['Wc'][i].astype(np.float32).T        # (288,48)
        wc12[i, 0] = _gap(WcT[0:96])
        wc12[i, 1] = _gap(WcT[96:192])
        wc3[i] = WcT[192:288]
        wr[i] = _gap(ins['Wr'][i].astype(np.float32).T)
    bc = ins['bc'].astype(np.float32).reshape(L - 1, H, 1)
    br = ins['br'].astype(np.float32).reshape(L - 1, H, 1)
    fclw = _gap(ins['fcl_w'].astype(np.float32).reshape(FM, 1))
    ident = np.zeros((128, 128), np.float32)
    ident[:64, :64] = np.eye(64)
    ident[64:112, 0:48] = np.eye(48)
    scal = np.array([[float(ins['fcl_w'].sum()),
                      float(ins['fcl_b'].reshape(-1)[0]), 0.0, 0.0]],
                    np.float32)

    in_maps = []
    for k in range(NC):
        s = slice(k * BPC, (k + 1) * BPC)
        in_maps.append({
            'emb_in': np.ascontiguousarray(emb_in[s]),
            'noderows': np.ascontiguousarray(noderows[s]),
            'node_colm': np.ascontiguousarray(node_colm[s]),
            'invnb': np.ascontiguousarray(invnb[s]),
            'Wc0T': wc0, 'Wr0T': wr0, 'bc0': bc0, 'br0': br0,
            'WcT12': wc12, 'WcT3': wc3, 'WrT': wr, 'bc': bc, 'br': br,
            'fclw': fclw, 'ident': ident, 'scal': scal,
        })

    global _LAST_INMAPS
    _LAST_INMAPS = in_maps
    runner = _get_runner(tuple(float(a) for a in alphas))
    outs = runner(in_maps)
    out = np.concatenate([outs[k].reshape(BPC) for k in range(NC)])
    return out.astype(np.float32)


def _get_runner(key, reps=1):
    """Persistent jitted SPMD executor (avoids per-call jax retracing)."""
    ck = (key, reps)
    if ck in _RUN_CACHE:
        return _RUN_CACHE[ck]
    import jax
    from jax.experimental.shard_map import shard_map
    from jax.sharding import Mesh, PartitionSpec
    from concourse import bass2jax, mybir as _mb

    if ck not in _CACHE:
        _CACHE[ck] = build_program(list(key), reps)
    nc_prog = _CACHE[ck]
    bass2jax.install_neuronx_cc_hook()

    pname = (nc_prog.partition_id_tensor.name
             if nc_prog.partition_id_tensor else None)
    in_names, out_names, out_avals, zero_outs = [], [], [], []
    for alloc in nc_prog.m.functions[0].allocations:
        if not isinstance(alloc, _mb.MemoryLocationSet):
            continue
        name = alloc.memorylocations[0].name
        if alloc.kind == 'ExternalInput':
            if name != pname:
                in_names.append(name)
        elif alloc.kind == 'ExternalOutput':
            out_names.append(name)
            shape = tuple(alloc.tensor_shape)
            dtype = _mb.dt.np(alloc.dtype)
            out_avals.append(jax.core.ShapedArray(shape, dtype))
            zero_outs.append(np.zeros(shape, dtype))
    n_params = len(in_names)
    all_names = in_names + out_names + ([pname] if pname else [])

    def _body(*args):
        operands = list(args)
        if pname:
            operands.append(bass2jax.partition_id_tensor())
        outs = bass2jax._bass_exec_p.bind(
            *operands, out_avals=tuple(out_avals), in_names=tuple(all_names),
            out_names=tuple(out_names), lowering_input_output_aliases=(),
            sim_require_finite=True, sim_require_nnan=True, nc=nc_prog)
        return tuple(outs)

    devices = jax.devices()[:NC]
    mesh = Mesh(np.asarray(devices), ('core',))
    n_outs = len(out_names)
    sharded = jax.jit(
        shard_map(_body, mesh=mesh,
                  in_specs=(PartitionSpec('core'),) * (n_params + n_outs),
                  out_specs=(PartitionSpec('core'),) * n_outs,
                  check_rep=False),
        keep_unused=True)

    def run(in_maps):
        concat_in = [np.concatenate([np.asarray(m[nm]) for m in in_maps],
                                    axis=0) for nm in in_names]
        concat_zero = [np.zeros((NC * z.shape[0], *z.shape[1:]), z.dtype)
                       for z in zero_outs]
        out_arrs = sharded(*concat_in, *concat_zero)
        o = np.asarray(out_arrs[0]).reshape(NC, *out_avals[0].shape)
        return [o[c] for c in range(NC)]

    _RUN_CACHE[ck] = run
    return run


if __name__ == '__main__':
    sys.path.insert(0, '/root/problem')
    import jax
    import reference as R
    cpu = jax.devices('cpu')[0]
    with jax.default_device(cpu):
        inp = {k: np.asarray(v) for k, v in R.setup_inputs().items()}
        exp = np.asarray(R.reference(**R.setup_inputs()))
    got = kernel(**inp)
    rel = np.abs(got - exp) / (np.abs(exp) + 1e-9)
    print('expected:', exp[:8])
    print('got     :', got[:8])
    print('max rel err:', rel.max())
